# revision 4
# baseline (speedup 1.0000x reference)
"""GNN NodeUpdateNetwork kernel for 8x Trainium2 NeuronCores.

Math (per task t):
    masked  = edge * (1 - I)                      # zero diagonal
    denom   = max(sum(masked, -1), 1e-12)         # L1 row norms (edge >= 0)
    aggr_e  = (masked_e @ node) / denom_e         # [N, D] per edge channel
    x       = [node | aggr_0 | aggr_1]            # [N, 3D]
    out     = lrelu(lrelu(x @ w0.T) @ w1.T)       # [N, OUT]

Sharding: core = (t, row-half). Each core handles 2048 output rows for one
task, both edge channels. Host passes a transposed ("m on partitions") and
rolled edge slice so that:
  - the PE contraction dim (m) lands on SBUF partitions with fully
    contiguous DMA loads,
  - the diagonal blocks sit at identical tile coordinates on every core
    (SPMD: one program for all 8 cores).
The ones-column prepended to node_ext makes psum row 0 the L1 row sums.
"""

import os
import time

import numpy as np

T, N, D, E, OUT = 4, 4096, 64, 2, 64
H0 = 2 * OUT               # 128
NH = N // 2                # 2048 rows per core
NCORES = 8
EPS = 1e-12
SLOPE = 0.01

CHUNK = 512                # psum free-dim chunk (one fp32 bank)
NJ = NH // CHUNK           # 4
MT = N // 128              # 32 m-tiles
G = 4                      # m-tiles per DMA call (4 MiB)
NG = MT // G               # 8

_PROGRAM = None


def _build_program():
    from contextlib import ExitStack

    import concourse.mybir as mybir
    import concourse.tile as tile
    from concourse import bacc

    fp32 = mybir.dt.float32
    nc = bacc.Bacc("TRN2", target_bir_lowering=False, debug=False)

    edgeT = nc.dram_tensor("edgeT", [E, N, NH], fp32, kind="ExternalInput")
    node_ext = nc.dram_tensor("node_ext", [N, 1 + D], fp32, kind="ExternalInput")
    nodeT_s = nc.dram_tensor("nodeT_s", [D, NH], fp32, kind="ExternalInput")
    w0ta = nc.dram_tensor("w0ta", [D, H0], fp32, kind="ExternalInput")
    w0tm = nc.dram_tensor("w0tm", [1 + D, H0], fp32, kind="ExternalInput")
    w0tb = nc.dram_tensor("w0tb", [1 + D, H0], fp32, kind="ExternalInput")
    w1t = nc.dram_tensor("w1t", [H0, OUT], fp32, kind="ExternalInput")
    dmask = nc.dram_tensor("dmask", [128, 128], fp32, kind="ExternalInput")
    outT = nc.dram_tensor("outT", [OUT, NH], fp32, kind="ExternalOutput")

    with tile.TileContext(nc) as tc, ExitStack() as ctx:
        singles = ctx.enter_context(tc.tile_pool(name="singles", bufs=1))
        edges = ctx.enter_context(tc.tile_pool(name="edges", bufs=2))
        smalls = ctx.enter_context(tc.tile_pool(name="smalls", bufs=2))
        paggr = ctx.enter_context(tc.tile_pool(name="paggr", bufs=1, space="PSUM"))
        pmlp = ctx.enter_context(tc.tile_pool(name="pmlp", bufs=3, space="PSUM"))

        # ---- constants / small inputs ----
        node_ext_sb = singles.tile([128, MT, 1 + D], fp32)
        nc.sync.dma_start(
            node_ext_sb, node_ext.ap().rearrange("(mt p) d -> p mt d", p=128)
        )
        nodeT_sb = singles.tile([D, NH], fp32)
        nc.sync.dma_start(nodeT_sb, nodeT_s.ap())
        w0ta_sb = singles.tile([D, H0], fp32)
        nc.sync.dma_start(w0ta_sb, w0ta.ap())
        w0tm_sb = singles.tile([1 + D, H0], fp32)
        nc.sync.dma_start(w0tm_sb, w0tm.ap())
        w0tb_sb = singles.tile([1 + D, H0], fp32)
        nc.sync.dma_start(w0tb_sb, w0tb.ap())
        w1t_sb = singles.tile([H0, OUT], fp32)
        nc.sync.dma_start(w1t_sb, w1t.ap())
        dmask_sb = singles.tile([128, 128], fp32)
        nc.sync.dma_start(dmask_sb, dmask.ap())
        ones_sb = singles.tile([1, 1 + D], fp32)
        nc.vector.memset(ones_sb, 1.0)

        xTm_sb = singles.tile([1 + D, NH], fp32)   # normalized aggr (e=0), row 0 junk
        xTb_sb = singles.tile([1 + D, NH], fp32)   # normalized aggr (e=1), row 0 junk
        outT_sb = singles.tile([OUT, NH], fp32)

        # ---- aggregation per edge channel ----
        for e in range(E):
            # psum rows: 0 = L1 row sums (ones column), 1..64 = raw aggregate
            psum_aggr = paggr.tile([1 + D, NH], fp32, tag="aggr")
            for g in range(NG):
                et = edges.tile([128, G, NH], fp32, tag="edge")
                nc.sync.dma_start(
                    et,
                    edgeT.ap()[e, 128 * G * g : 128 * G * (g + 1), :].rearrange(
                        "(k p) n -> p k n", p=128
                    ),
                )
                for k in range(G):
                    mt = G * g + k
                    if mt < 16:
                        # this m-tile crosses the diagonal: zero it
                        sl = et[:, k, 128 * mt : 128 * (mt + 1)]
                        nc.vector.tensor_mul(sl, sl, dmask_sb)
                    for j in range(NJ):
                        nc.tensor.matmul(
                            psum_aggr[:, CHUNK * j : CHUNK * (j + 1)],
                            node_ext_sb[:, mt, :],
                            et[:, k, CHUNK * j : CHUNK * (j + 1)],
                            start=(mt == 0),
                            stop=(mt == MT - 1),
                        )
            aggr_sb = smalls.tile([1 + D, NH], fp32, tag="aggr_sb")
            nc.scalar.copy(aggr_sb, psum_aggr)
            dn = smalls.tile([1, NH], fp32, tag="dn", bufs=1)
            nc.vector.tensor_scalar_max(dn, aggr_sb[0:1, :], EPS)
            inv = smalls.tile([1, NH], fp32, tag="inv", bufs=1)
            nc.vector.reciprocal(inv, dn)
            dest = xTm_sb if e == 0 else xTb_sb
            for j in range(NJ):
                sl = slice(CHUNK * j, CHUNK * (j + 1))
                pb = pmlp.tile([1 + D, CHUNK], fp32, tag="mlp")
                nc.tensor.matmul(pb, ones_sb, inv[:, sl], start=True, stop=True)
                inv_bc = smalls.tile([1 + D, CHUNK], fp32, tag="invbc")
                nc.scalar.copy(inv_bc, pb)
                nc.vector.tensor_mul(dest[:, sl], aggr_sb[:, sl], inv_bc)

        # ---- per-node MLP (feature-major) ----
        # leaky_relu(x) = max(0.01*x, x), composed from ACT mul + DVE max
        for j in range(NJ):
            sl = slice(CHUNK * j, CHUNK * (j + 1))
            ph = pmlp.tile([H0, CHUNK], fp32, tag="mlp")
            nc.tensor.matmul(ph, w0ta_sb, nodeT_sb[:, sl], start=True, stop=False)
            nc.tensor.matmul(ph, w0tm_sb, xTm_sb[:, sl], start=False, stop=False)
            nc.tensor.matmul(ph, w0tb_sb, xTb_sb[:, sl], start=False, stop=True)
            hs = smalls.tile([H0, CHUNK], fp32, tag="hs")
            nc.scalar.mul(hs, ph, SLOPE)
            hT = smalls.tile([H0, CHUNK], fp32, tag="hT")
            nc.vector.tensor_max(hT, hs, ph)
            po = pmlp.tile([OUT, CHUNK], fp32, tag="mlp")
            nc.tensor.matmul(po, w1t_sb, hT, start=True, stop=True)
            os_ = smalls.tile([OUT, CHUNK], fp32, tag="os")
            nc.scalar.mul(os_, po, SLOPE)
            nc.vector.tensor_max(outT_sb[:, sl], os_, po)

        nc.sync.dma_start(outT.ap(), outT_sb)

    nc.compile()
    return nc


def _get_program():
    global _PROGRAM
    if _PROGRAM is None:
        _PROGRAM = _build_program()
    return _PROGRAM


def _prep_inputs(node_feat, edge_feat, w0, w1):
    """Per-core input maps. Layout-only host work (transpose/roll/concat)."""
    node_feat = np.ascontiguousarray(node_feat, dtype=np.float32)
    edge_feat = np.ascontiguousarray(edge_feat, dtype=np.float32)
    w0 = np.ascontiguousarray(w0, dtype=np.float32)
    w1 = np.ascontiguousarray(w1, dtype=np.float32)

    w0ta = np.ascontiguousarray(w0[:, 0:D].T)                       # [64, 128]
    zrow = np.zeros((1, H0), np.float32)
    w0tm = np.ascontiguousarray(
        np.concatenate([zrow, w0[:, D : 2 * D].T], axis=0))         # [65, 128]
    w0tb = np.ascontiguousarray(
        np.concatenate([zrow, w0[:, 2 * D : 3 * D].T], axis=0))     # [65, 128]
    w1t = np.ascontiguousarray(w1.T)                                # [128, 64]
    dmask = np.ascontiguousarray(
        (1.0 - np.eye(128)).astype(np.float32))                     # [128, 128]
    ones_col = np.ones((N, 1), np.float32)

    in_maps = []
    for core in range(NCORES):
        t, half = divmod(core, 2)
        r0 = half * NH
        # edgeT[e, m', nl] = edge[t, e, r0+nl, (m'+r0) % N]
        subT = edge_feat[t, :, r0 : r0 + NH, :].transpose(0, 2, 1)  # [E, N, NH]
        edgeT = np.ascontiguousarray(
            np.concatenate([subT[:, r0:, :], subT[:, :r0, :]], axis=1)
        )
        # node_ext[m', :] = [1 | node[t, (m'+r0) % N, :]]
        ne = np.concatenate([ones_col, node_feat[t]], axis=1)       # [N, 65]
        node_ext = np.ascontiguousarray(
            np.concatenate([ne[r0:], ne[:r0]], axis=0))
        nodeT_s = np.ascontiguousarray(node_feat[t, r0 : r0 + NH, :].T)
        in_maps.append(
            {
                "edgeT": edgeT,
                "node_ext": node_ext,
                "nodeT_s": nodeT_s,
                "w0ta": w0ta,
                "w0tm": w0tm,
                "w0tb": w0tb,
                "w1t": w1t,
                "dmask": dmask,
            }
        )
    return in_maps


def _install_ntff_hook():
    """Recreate the missing antenv.axon_hooks shim so trace=True can capture
    NTFF profiles through libaxon_pjrt (profiling only; unused when grading)."""
    import sys
    import types

    if "antenv.axon_hooks" in sys.modules:
        return
    try:
        from trn_agent_boot.trn_boot import _ntff_profile_via_ctypes
    except ImportError:
        return
    mod = types.ModuleType("antenv.axon_hooks")
    hook = _ntff_profile_via_ctypes("/opt/axon/libaxon_pjrt.so")
    mod._hook = hook
    mod.set_axon_ntff_profile_hook = lambda h: setattr(mod, "_hook", h)
    mod.get_axon_ntff_profile_hook = lambda: mod._hook
    sys.modules["antenv.axon_hooks"] = mod


def kernel(node_feat, edge_feat, w0, w1):
    from concourse import bass_utils

    in_maps = _prep_inputs(node_feat, edge_feat, w0, w1)
    nc = _get_program()

    trace = bool(int(os.environ.get("GNN_TRACE", "0")))
    if trace:
        _install_ntff_hook()
    t0 = time.time()
    res = bass_utils.run_bass_kernel_spmd(
        nc,
        in_maps,
        core_ids=list(range(NCORES)),
        trace=trace,
        trace_cores=list(range(NCORES)) if trace else None,
    )
    wall = time.time() - t0
    if trace:
        print(f"kernel wall time: {wall * 1e9:.0f} ns")
        if res.exec_time_ns is not None:
            print(f"HW exec time: {res.exec_time_ns} ns")
            print(f"HW exec time mean: {res.mean_exec_time_ns} ns")
            print(f"slowest core: {res.max_exec_time_core_id}")
        if res.instructions_and_trace is not None:
            print(f"trace: {res.instructions_and_trace[1]}")

    out = np.empty((T, N, OUT), np.float32)
    for core in range(NCORES):
        t, half = divmod(core, 2)
        out[t, half * NH : (half + 1) * NH, :] = res.results[core]["outT"].T
    return out


# revision 10
# speedup vs baseline: 1.1948x; 1.1948x over previous
"""GNN NodeUpdateNetwork kernel for 8x Trainium2 NeuronCores.

Math (per task t):
    masked  = edge * (1 - I)                      # zero diagonal
    denom   = max(sum(masked, -1), 1e-12)         # L1 row norms (edge >= 0)
    aggr_e  = (masked_e @ node) / denom_e         # [N, D] per edge channel
    x       = [node | aggr_0 | aggr_1]            # [N, 3D]
    out     = lrelu(lrelu(x @ w0.T) @ w1.T)       # [N, OUT]

Sharding: core = (t, row-half). Each core handles 2048 output rows for one
task, both edge channels. Host passes a transposed ("m on partitions") and
rolled edge slice so that:
  - the PE contraction dim (m) lands on SBUF partitions with fully
    contiguous DMA loads,
  - the diagonal blocks sit at identical tile coordinates on every core
    (SPMD: one program for all 8 cores).
The ones-column prepended to node_ext makes psum row 0 the L1 row sums.
"""

import os
import time

import numpy as np

T, N, D, E, OUT = 4, 4096, 64, 2, 64
H0 = 2 * OUT               # 128
NH = N // 2                # 2048 rows per core
NCORES = 8
EPS = 1e-12
SLOPE = 0.01

CHUNK = 512                # psum free-dim chunk (one fp32 bank)
NJ = NH // CHUNK           # 4
MT = N // 128              # 32 m-tiles
G = 4                      # m-tiles per DMA call (4 MiB)
NG = MT // G               # 8

_PROGRAM = None


def _build_program():
    from contextlib import ExitStack

    import concourse.mybir as mybir
    import concourse.tile as tile
    from concourse import bacc

    fp32 = mybir.dt.float32
    # aggregation matmul input mode:
    #   fp32  = exact (2 HW passes per matmul)
    #   fp32r = single-pass reduced-precision fp32 (SWDGE DMA rounds on load)
    #   bf16  = single-pass bf16 (SWDGE DMA casts on load)
    mode = os.environ.get("GNN_MM_DTYPE", "fp32r")
    mm_dt = {
        "fp32": fp32,
        "fp32r": mybir.dt.float32r,
        "bf16": mybir.dt.bfloat16,
    }[mode]
    cast_load = mode != "fp32"
    nc = bacc.Bacc("TRN2", target_bir_lowering=False, debug=False)

    edgeT = nc.dram_tensor("edgeT", [E, N, NH], fp32, kind="ExternalInput")
    node_ext = nc.dram_tensor("node_ext", [N, 1 + D], fp32, kind="ExternalInput")
    nodeT_s = nc.dram_tensor("nodeT_s", [D, NH], fp32, kind="ExternalInput")
    w0ta = nc.dram_tensor("w0ta", [D, H0], fp32, kind="ExternalInput")
    w0tm = nc.dram_tensor("w0tm", [1 + D, H0], fp32, kind="ExternalInput")
    w0tb = nc.dram_tensor("w0tb", [1 + D, H0], fp32, kind="ExternalInput")
    w1t = nc.dram_tensor("w1t", [H0, OUT], fp32, kind="ExternalInput")
    dmask = nc.dram_tensor("dmask", [128, 128], fp32, kind="ExternalInput")
    outT = nc.dram_tensor("outT", [OUT, NH], fp32, kind="ExternalOutput")

    with tile.TileContext(nc) as tc, ExitStack() as ctx:
        singles = ctx.enter_context(tc.tile_pool(name="singles", bufs=1))
        edges = ctx.enter_context(tc.tile_pool(name="edges", bufs=3))
        smalls = ctx.enter_context(tc.tile_pool(name="smalls", bufs=2))
        paggr = ctx.enter_context(tc.tile_pool(name="paggr", bufs=1, space="PSUM"))
        pmlp = ctx.enter_context(tc.tile_pool(name="pmlp", bufs=3, space="PSUM"))

        # ---- constants / small inputs ----
        node_ext_sb = singles.tile([128, MT, 1 + D], mm_dt)
        _ne_dma = nc.gpsimd if cast_load else nc.sync
        _ne_dma.dma_start(
            node_ext_sb, node_ext.ap().rearrange("(mt p) d -> p mt d", p=128)
        )
        nodeT_sb = singles.tile([D, NH], fp32)
        nc.sync.dma_start(nodeT_sb, nodeT_s.ap())
        w0ta_sb = singles.tile([D, H0], fp32)
        nc.sync.dma_start(w0ta_sb, w0ta.ap())
        w0tm_sb = singles.tile([1 + D, H0], fp32)
        nc.sync.dma_start(w0tm_sb, w0tm.ap())
        w0tb_sb = singles.tile([1 + D, H0], fp32)
        nc.sync.dma_start(w0tb_sb, w0tb.ap())
        w1t_sb = singles.tile([H0, OUT], fp32)
        nc.sync.dma_start(w1t_sb, w1t.ap())
        dmask_sb = singles.tile([128, 128], fp32)
        nc.sync.dma_start(dmask_sb, dmask.ap())
        ones_sb = singles.tile([1, 1 + D], fp32)
        nc.vector.memset(ones_sb, 1.0)

        xTm_sb = singles.tile([1 + D, NH], fp32)   # normalized aggr (e=0), row 0 junk
        xTb_sb = singles.tile([1 + D, NH], fp32)   # normalized aggr (e=1), row 0 junk
        outT_sb = singles.tile([OUT, NH], fp32)

        # ---- aggregation per edge channel ----
        for e in range(E):
            # psum rows: 0 = L1 row sums (ones column), 1..64 = raw aggregate
            psum_aggr = paggr.tile([1 + D, NH], fp32, tag="aggr")
            for g in range(NG):
                et = edges.tile([128, G, NH], mm_dt, tag="edge")
                (nc.gpsimd if cast_load else nc.sync).dma_start(
                    et,
                    edgeT.ap()[e, 128 * G * g : 128 * G * (g + 1), :].rearrange(
                        "(k p) n -> p k n", p=128
                    ),
                )
                for k in range(G):
                    mt = G * g + k
                    if mt < 16:
                        # this m-tile crosses the diagonal: zero it
                        sl = et[:, k, 128 * mt : 128 * (mt + 1)]
                        nc.vector.tensor_mul(sl, sl, dmask_sb)
                    for j in range(NJ):
                        nc.tensor.matmul(
                            psum_aggr[:, CHUNK * j : CHUNK * (j + 1)],
                            node_ext_sb[:, mt, :],
                            et[:, k, CHUNK * j : CHUNK * (j + 1)],
                            start=(mt == 0),
                            stop=(mt == MT - 1),
                        )
            aggr_sb = smalls.tile([1 + D, NH], fp32, tag="aggr_sb")
            nc.scalar.copy(aggr_sb, psum_aggr)
            dn = smalls.tile([1, NH], fp32, tag="dn", bufs=1)
            nc.vector.tensor_scalar_max(dn, aggr_sb[0:1, :], EPS)
            inv = smalls.tile([1, NH], fp32, tag="inv", bufs=1)
            nc.vector.reciprocal(inv, dn)
            dest = xTm_sb if e == 0 else xTb_sb
            for j in range(NJ):
                sl = slice(CHUNK * j, CHUNK * (j + 1))
                pb = pmlp.tile([1 + D, CHUNK], fp32, tag="mlp")
                nc.tensor.matmul(pb, ones_sb, inv[:, sl], start=True, stop=True)
                inv_bc = smalls.tile([1 + D, CHUNK], fp32, tag="invbc")
                nc.scalar.copy(inv_bc, pb)
                nc.vector.tensor_mul(dest[:, sl], aggr_sb[:, sl], inv_bc)

        # ---- per-node MLP (feature-major) ----
        # leaky_relu(x) = max(0.01*x, x), composed from ACT mul + DVE max
        for j in range(NJ):
            sl = slice(CHUNK * j, CHUNK * (j + 1))
            ph = pmlp.tile([H0, CHUNK], fp32, tag="mlp")
            nc.tensor.matmul(ph, w0ta_sb, nodeT_sb[:, sl], start=True, stop=False)
            nc.tensor.matmul(ph, w0tm_sb, xTm_sb[:, sl], start=False, stop=False)
            nc.tensor.matmul(ph, w0tb_sb, xTb_sb[:, sl], start=False, stop=True)
            hs = smalls.tile([H0, CHUNK], fp32, tag="hs")
            nc.scalar.mul(hs, ph, SLOPE)
            hT = smalls.tile([H0, CHUNK], fp32, tag="hT")
            nc.vector.tensor_max(hT, hs, ph)
            po = pmlp.tile([OUT, CHUNK], fp32, tag="mlp")
            nc.tensor.matmul(po, w1t_sb, hT, start=True, stop=True)
            os_ = smalls.tile([OUT, CHUNK], fp32, tag="os")
            nc.scalar.mul(os_, po, SLOPE)
            nc.vector.tensor_max(outT_sb[:, sl], os_, po)

        nc.sync.dma_start(outT.ap(), outT_sb)

    nc.compile()
    return nc


def _get_program():
    global _PROGRAM
    if _PROGRAM is None:
        _PROGRAM = _build_program()
    return _PROGRAM


def _prep_inputs(node_feat, edge_feat, w0, w1):
    """Per-core input maps. Layout-only host work (transpose/roll/concat)."""
    node_feat = np.ascontiguousarray(node_feat, dtype=np.float32)
    edge_feat = np.ascontiguousarray(edge_feat, dtype=np.float32)
    w0 = np.ascontiguousarray(w0, dtype=np.float32)
    w1 = np.ascontiguousarray(w1, dtype=np.float32)

    w0ta = np.ascontiguousarray(w0[:, 0:D].T)                       # [64, 128]
    zrow = np.zeros((1, H0), np.float32)
    w0tm = np.ascontiguousarray(
        np.concatenate([zrow, w0[:, D : 2 * D].T], axis=0))         # [65, 128]
    w0tb = np.ascontiguousarray(
        np.concatenate([zrow, w0[:, 2 * D : 3 * D].T], axis=0))     # [65, 128]
    w1t = np.ascontiguousarray(w1.T)                                # [128, 64]
    dmask = np.ascontiguousarray(
        (1.0 - np.eye(128)).astype(np.float32))                     # [128, 128]
    ones_col = np.ones((N, 1), np.float32)

    in_maps = []
    for core in range(NCORES):
        t, half = divmod(core, 2)
        r0 = half * NH
        # edgeT[e, m', nl] = edge[t, e, r0+nl, (m'+r0) % N]
        subT = edge_feat[t, :, r0 : r0 + NH, :].transpose(0, 2, 1)  # [E, N, NH]
        edgeT = np.ascontiguousarray(
            np.concatenate([subT[:, r0:, :], subT[:, :r0, :]], axis=1)
        )
        # node_ext[m', :] = [1 | node[t, (m'+r0) % N, :]]
        ne = np.concatenate([ones_col, node_feat[t]], axis=1)       # [N, 65]
        node_ext = np.ascontiguousarray(
            np.concatenate([ne[r0:], ne[:r0]], axis=0))
        nodeT_s = np.ascontiguousarray(node_feat[t, r0 : r0 + NH, :].T)
        in_maps.append(
            {
                "edgeT": edgeT,
                "node_ext": node_ext,
                "nodeT_s": nodeT_s,
                "w0ta": w0ta,
                "w0tm": w0tm,
                "w0tb": w0tb,
                "w1t": w1t,
                "dmask": dmask,
            }
        )
    return in_maps


def _install_ntff_hook():
    """Recreate the missing antenv.axon_hooks shim so trace=True can capture
    NTFF profiles through libaxon_pjrt (profiling only; unused when grading)."""
    import sys
    import types

    if "antenv.axon_hooks" in sys.modules:
        return
    try:
        from trn_agent_boot.trn_boot import _ntff_profile_via_ctypes
    except ImportError:
        return
    mod = types.ModuleType("antenv.axon_hooks")
    hook = _ntff_profile_via_ctypes("/opt/axon/libaxon_pjrt.so")
    mod._hook = hook
    mod.set_axon_ntff_profile_hook = lambda h: setattr(mod, "_hook", h)
    mod.get_axon_ntff_profile_hook = lambda: mod._hook
    sys.modules["antenv.axon_hooks"] = mod


def kernel(node_feat, edge_feat, w0, w1):
    from concourse import bass_utils

    in_maps = _prep_inputs(node_feat, edge_feat, w0, w1)
    nc = _get_program()

    trace = bool(int(os.environ.get("GNN_TRACE", "0")))
    if trace:
        _install_ntff_hook()
    t0 = time.time()
    res = bass_utils.run_bass_kernel_spmd(
        nc,
        in_maps,
        core_ids=list(range(NCORES)),
        trace=trace,
        trace_cores=list(range(NCORES)) if trace else None,
    )
    wall = time.time() - t0
    if trace:
        print(f"kernel wall time: {wall * 1e9:.0f} ns")
        if res.exec_time_ns is not None:
            print(f"HW exec time: {res.exec_time_ns} ns")
            print(f"HW exec time mean: {res.mean_exec_time_ns} ns")
            print(f"slowest core: {res.max_exec_time_core_id}")
        if res.instructions_and_trace is not None:
            print(f"trace: {res.instructions_and_trace[1]}")

    out = np.empty((T, N, OUT), np.float32)
    for core in range(NCORES):
        t, half = divmod(core, 2)
        out[t, half * NH : (half + 1) * NH, :] = res.results[core]["outT"].T
    return out


# revision 15
# speedup vs baseline: 1.3081x; 1.0948x over previous
"""GNN NodeUpdateNetwork kernel for 8x Trainium2 NeuronCores.

Math (per task t):
    masked  = edge * (1 - I)                      # zero diagonal
    denom   = max(sum(masked, -1), 1e-12)         # L1 row norms (edge >= 0)
    aggr_e  = (masked_e @ node) / denom_e         # [N, D] per edge channel
    x       = [node | aggr_0 | aggr_1]            # [N, 3D]
    out     = lrelu(lrelu(x @ w0.T) @ w1.T)       # [N, OUT]

Sharding: core = (t, row-half). Each core handles 2048 output rows for one
task, both edge channels. Host passes a transposed ("m on partitions") and
rolled edge slice so that:
  - the PE contraction dim (m) lands on SBUF partitions with fully
    contiguous DMA loads,
  - the diagonal blocks sit at identical tile coordinates on every core
    (SPMD: one program for all 8 cores).
The ones-column prepended to node_ext makes psum row 0 the L1 row sums.

Matmuls run in float32r (single-pass fp32, ~1e-5 precision) so that even a
HAM-throttled (1.2 GHz) PE keeps up with the HBM stream; the kernel is
DMA-bound end to end.
"""

import os
import time

import numpy as np

T, N, D, E, OUT = 4, 4096, 64, 2, 64
H0 = 2 * OUT               # 128
NH = N // 2                # 2048 rows per core
NCORES = 8
EPS = 1e-12
SLOPE = 0.01

CHUNK = 512                # psum free-dim chunk (one fp32 bank)
NJ = NH // CHUNK           # 4
MT = N // 128              # 32 m-tiles
G = 4                      # m-tiles per DMA call (4 MiB)
NG = MT // G               # 8

_PROGRAM = None


def _build_program():
    from contextlib import ExitStack

    import concourse.mybir as mybir
    import concourse.tile as tile
    from concourse import bacc

    fp32 = mybir.dt.float32
    # matmul input mode: fp32 = exact 2-pass; fp32r = single-pass fp32
    mode = os.environ.get("GNN_MM_DTYPE", "fp32r")
    mm_dt = {
        "fp32": fp32,
        "fp32r": mybir.dt.float32r,
        "bf16": mybir.dt.bfloat16,
    }[mode]
    # bf16 needs a casting (SWDGE) load; fp32/fp32r stream bytes untouched
    cast_load = mode == "bf16"
    io_dt = fp32 if cast_load else mm_dt

    nc = bacc.Bacc("TRN2", target_bir_lowering=False, debug=False)

    edgeT = nc.dram_tensor("edgeT", [E, N, NH], io_dt, kind="ExternalInput")
    node_ext = nc.dram_tensor("node_ext", [N, 1 + D], io_dt, kind="ExternalInput")
    nodeT_s = nc.dram_tensor("nodeT_s", [D, NH], io_dt, kind="ExternalInput")
    w0ta = nc.dram_tensor("w0ta", [D, H0], io_dt, kind="ExternalInput")
    w0tm = nc.dram_tensor("w0tm", [1 + D, H0], io_dt, kind="ExternalInput")
    w0tb = nc.dram_tensor("w0tb", [1 + D, H0], io_dt, kind="ExternalInput")
    w1t = nc.dram_tensor("w1t", [H0, OUT], io_dt, kind="ExternalInput")
    dmask = nc.dram_tensor("dmask", [128, 128], fp32, kind="ExternalInput")
    ones1 = nc.dram_tensor("ones1", [1, 1 + D], io_dt, kind="ExternalInput")
    outT = nc.dram_tensor("outT", [OUT, NH], fp32, kind="ExternalOutput")

    with tile.TileContext(nc) as tc, ExitStack() as ctx:
        singles = ctx.enter_context(tc.tile_pool(name="singles", bufs=1))
        edges = ctx.enter_context(tc.tile_pool(name="edges", bufs=3))
        smalls = ctx.enter_context(tc.tile_pool(name="smalls", bufs=2))
        paggr = ctx.enter_context(tc.tile_pool(name="paggr", bufs=1, space="PSUM"))
        pmlp = ctx.enter_context(tc.tile_pool(name="pmlp", bufs=3, space="PSUM"))

        ldma = nc.gpsimd if cast_load else nc.sync

        # ---- constants / small inputs ----
        node_ext_sb = singles.tile([128, MT, 1 + D], mm_dt)
        ldma.dma_start(
            node_ext_sb, node_ext.ap().rearrange("(mt p) d -> p mt d", p=128)
        )
        nodeT_sb = singles.tile([D, NH], mm_dt)
        ldma.dma_start(nodeT_sb, nodeT_s.ap())
        w0ta_sb = singles.tile([D, H0], mm_dt)
        ldma.dma_start(w0ta_sb, w0ta.ap())
        w0tm_sb = singles.tile([1 + D, H0], mm_dt)
        ldma.dma_start(w0tm_sb, w0tm.ap())
        w0tb_sb = singles.tile([1 + D, H0], mm_dt)
        ldma.dma_start(w0tb_sb, w0tb.ap())
        w1t_sb = singles.tile([H0, OUT], mm_dt)
        ldma.dma_start(w1t_sb, w1t.ap())
        dmask_sb = singles.tile([128, 128], fp32)
        nc.sync.dma_start(dmask_sb, dmask.ap())
        ones_sb = singles.tile([1, 1 + D], mm_dt)
        ldma.dma_start(ones_sb, ones1.ap())

        xTm_sb = singles.tile([1 + D, NH], mm_dt)  # normalized aggr (e=0), row 0 junk
        xTb_sb = singles.tile([1 + D, NH], mm_dt)  # normalized aggr (e=1), row 0 junk
        outT_sb = singles.tile([OUT, NH], fp32)

        # ---- aggregation per edge channel, fused normalize (+MLP on e=1) ----
        for e in range(E):
            # psum rows: 0 = L1 row sums (ones column), 1..64 = raw aggregate
            psum_aggr = paggr.tile([1 + D, NH], fp32, tag="aggr")
            for g in range(NG):
                et = edges.tile([128, G, NH], mm_dt, tag="edge")
                ldma.dma_start(
                    et,
                    edgeT.ap()[e, 128 * G * g : 128 * G * (g + 1), :].rearrange(
                        "(k p) n -> p k n", p=128
                    ),
                )
                for k in range(G):
                    mt = G * g + k
                    if mt < 16:
                        # this m-tile crosses the diagonal: zero it
                        sl = et[:, k, 128 * mt : 128 * (mt + 1)]
                        nc.vector.tensor_mul(sl, sl, dmask_sb)
                    for j in range(NJ):
                        nc.tensor.matmul(
                            psum_aggr[:, CHUNK * j : CHUNK * (j + 1)],
                            node_ext_sb[:, mt, :],
                            et[:, k, CHUNK * j : CHUNK * (j + 1)],
                            start=(mt == 0),
                            stop=(mt == MT - 1),
                        )
            dest = xTm_sb if e == 0 else xTb_sb
            for j in range(NJ):
                sl = slice(CHUNK * j, CHUNK * (j + 1))
                # evacuate psum chunk early so the next channel can start
                aggr_sb = smalls.tile([1 + D, CHUNK], fp32, tag="aggr_sb", bufs=5)
                nc.scalar.copy(aggr_sb, psum_aggr[:, sl])
                dn = smalls.tile([1, CHUNK], fp32, tag="dn")
                nc.vector.tensor_scalar_max(dn, aggr_sb[0:1, :], EPS)
                inv = smalls.tile([1, CHUNK], mm_dt, tag="inv")
                with nc.allow_low_precision(reason="fp32r == fp32 bit width"):
                    nc.vector.reciprocal(inv, dn)
                pb = pmlp.tile([1 + D, CHUNK], fp32, tag="mlp")
                nc.tensor.matmul(pb, ones_sb, inv, start=True, stop=True)
                inv_bc = smalls.tile([1 + D, CHUNK], fp32, tag="invbc")
                nc.scalar.copy(inv_bc, pb)
                nc.vector.tensor_mul(dest[:, sl], aggr_sb, inv_bc)
                if e == 1:
                    # MLP chunk: leaky_relu(x) = max(0.01*x, x)
                    ph = pmlp.tile([H0, CHUNK], fp32, tag="mlp")
                    nc.tensor.matmul(
                        ph, w0ta_sb, nodeT_sb[:, sl], start=True, stop=False
                    )
                    nc.tensor.matmul(
                        ph, w0tm_sb, xTm_sb[:, sl], start=False, stop=False
                    )
                    nc.tensor.matmul(
                        ph, w0tb_sb, xTb_sb[:, sl], start=False, stop=True
                    )
                    hs = smalls.tile([H0, CHUNK], fp32, tag="hs")
                    nc.scalar.mul(hs, ph, SLOPE)
                    hT = smalls.tile([H0, CHUNK], mm_dt, tag="hT")
                    nc.vector.tensor_max(hT, hs, ph)
                    po = pmlp.tile([OUT, CHUNK], fp32, tag="mlp")
                    nc.tensor.matmul(po, w1t_sb, hT, start=True, stop=True)
                    os_ = smalls.tile([OUT, CHUNK], fp32, tag="os")
                    nc.scalar.mul(os_, po, SLOPE)
                    nc.vector.tensor_max(outT_sb[:, sl], os_, po)

        nc.sync.dma_start(outT.ap(), outT_sb)

    nc.compile()
    return nc


def _get_program():
    global _PROGRAM
    if _PROGRAM is None:
        _PROGRAM = _build_program()
    return _PROGRAM


def _prep_inputs(node_feat, edge_feat, w0, w1):
    """Per-core input maps. Layout-only host work (transpose/roll/concat)."""
    node_feat = np.ascontiguousarray(node_feat, dtype=np.float32)
    edge_feat = np.ascontiguousarray(edge_feat, dtype=np.float32)
    w0 = np.ascontiguousarray(w0, dtype=np.float32)
    w1 = np.ascontiguousarray(w1, dtype=np.float32)

    w0ta = np.ascontiguousarray(w0[:, 0:D].T)                       # [64, 128]
    zrow = np.zeros((1, H0), np.float32)
    w0tm = np.ascontiguousarray(
        np.concatenate([zrow, w0[:, D : 2 * D].T], axis=0))         # [65, 128]
    w0tb = np.ascontiguousarray(
        np.concatenate([zrow, w0[:, 2 * D : 3 * D].T], axis=0))     # [65, 128]
    w1t = np.ascontiguousarray(w1.T)                                # [128, 64]
    dmask = np.ascontiguousarray(
        (1.0 - np.eye(128)).astype(np.float32))                     # [128, 128]
    ones_col = np.ones((N, 1), np.float32)

    in_maps = []
    for core in range(NCORES):
        t, half = divmod(core, 2)
        r0 = half * NH
        # edgeT[e, m', nl] = edge[t, e, r0+nl, (m'+r0) % N]
        subT = edge_feat[t, :, r0 : r0 + NH, :].transpose(0, 2, 1)  # [E, N, NH]
        edgeT = np.ascontiguousarray(
            np.concatenate([subT[:, r0:, :], subT[:, :r0, :]], axis=1)
        )
        # node_ext[m', :] = [1 | node[t, (m'+r0) % N, :]]
        ne = np.concatenate([ones_col, node_feat[t]], axis=1)       # [N, 65]
        node_ext = np.ascontiguousarray(
            np.concatenate([ne[r0:], ne[:r0]], axis=0))
        nodeT_s = np.ascontiguousarray(node_feat[t, r0 : r0 + NH, :].T)
        in_maps.append(
            {
                "edgeT": edgeT,
                "node_ext": node_ext,
                "nodeT_s": nodeT_s,
                "w0ta": w0ta,
                "w0tm": w0tm,
                "w0tb": w0tb,
                "w1t": w1t,
                "dmask": dmask,
                "ones1": np.ones((1, 1 + D), np.float32),
            }
        )
    return in_maps


def _install_ntff_hook():
    """Recreate the missing antenv.axon_hooks shim so trace=True can capture
    NTFF profiles through libaxon_pjrt (profiling only; unused when grading)."""
    import sys
    import types

    if "antenv.axon_hooks" in sys.modules:
        return
    try:
        from trn_agent_boot.trn_boot import _ntff_profile_via_ctypes
    except ImportError:
        return
    mod = types.ModuleType("antenv.axon_hooks")
    hook = _ntff_profile_via_ctypes("/opt/axon/libaxon_pjrt.so")
    mod._hook = hook
    mod.set_axon_ntff_profile_hook = lambda h: setattr(mod, "_hook", h)
    mod.get_axon_ntff_profile_hook = lambda: mod._hook
    sys.modules["antenv.axon_hooks"] = mod


def kernel(node_feat, edge_feat, w0, w1):
    from concourse import bass_utils

    in_maps = _prep_inputs(node_feat, edge_feat, w0, w1)
    nc = _get_program()

    trace = bool(int(os.environ.get("GNN_TRACE", "0")))
    if trace:
        _install_ntff_hook()
    t0 = time.time()
    res = bass_utils.run_bass_kernel_spmd(
        nc,
        in_maps,
        core_ids=list(range(NCORES)),
        trace=trace,
        trace_cores=list(range(NCORES)) if trace else None,
    )
    wall = time.time() - t0
    if trace:
        print(f"kernel wall time: {wall * 1e9:.0f} ns")
        if res.exec_time_ns is not None:
            print(f"HW exec time: {res.exec_time_ns} ns")
            print(f"HW exec time mean: {res.mean_exec_time_ns} ns")
            print(f"slowest core: {res.max_exec_time_core_id}")
        if res.instructions_and_trace is not None:
            print(f"trace: {res.instructions_and_trace[1]}")

    out = np.empty((T, N, OUT), np.float32)
    for core in range(NCORES):
        t, half = divmod(core, 2)
        out[t, half * NH : (half + 1) * NH, :] = res.results[core]["outT"].T
    return out
